# revision 23
# baseline (speedup 1.0000x reference)
"""Multi-head attention (S=2048, B=2, D=1024, H=16) on 8 Trainium2 NeuronCores.

Sharding: batch x head-group. Core c handles batch c//4 and heads
[4*(c%4), 4*(c%4)+4). Each core computes its 4 heads' Q/K/V projections,
attention, and a partial output projection (row-parallel Wo); the host sums
the 4 partials per batch and adds the bias terms (bo and the exact wo@bv
correction; softmax rows sum to 1 so bv folds out of the attention).

v3 device-side structure (per core):
  - x inputs and all weights arrive as bf16 (host-converted), halving HBM
    traffic; projections run bf16 through the PE at 1 cycle/row.
  - Q/K stored f32r, dk-major: tile hp holds heads 2hp,2hp+1 at partition
    ranges [0,64) and [64,128); score matmuls use PE quadrant packing
    (tile_position=(hx*64,0)) with K=dk=64.
  - exp is split across two engines: ACT runs activation(Exp)->fp16 with
    the 1/sqrt(dk) scale folded in; DVE runs a Schraudolph-style
    round(s*K + B) -> int16 whose bit pattern IS fp16(exp(s*scale)) (~2%
    rms, cancels further through the shared softmax denominator). All other
    elementwise work (bias adds, conversions, 1/Z normalize) is greedily
    load-balanced between ACT and DVE.
  - P@V runs fp16 with a ones-column appended to V so the PE accumulates
    the softmax denominator Z next to U; normalization is deferred past
    P@V (one reciprocal + per-partition-scalar multiplies).
  - PSUM is partitioned to decouple the pipeline: score tiles get a
    dedicated 2-deep rotation (4 banks), the two live U accumulators get
    their own banks, and projection/transpose/output-projection scratch
    rotates in a third pool, so scores(t+1) never serialize behind exp(t).
  - P@V is emitted four ticks behind scores/exp so its matmuls never park
    in the PE wait queue (depth 4) blocking later scores; input slabs land
    as two half-DMAs so projections start after half the transfer.
  - O = U * (1/Z) -> bf16, PE-transposed, output projection in bf16;
    y partials fly to HBM as bf16 and the host sums them in f32.
"""

import sys

sys.path.insert(0, "/opt/trn_rl_repo")

from collections import deque

import numpy as np
import ml_dtypes

import concourse.bass as bass
import concourse.tile as tile
from concourse import bacc, mybir
from concourse.bass_utils import run_bass_kernel_spmd
from concourse.masks import make_identity

S = 2048
B = 2
D = 1024
H = 16
DK = 64
G = 4            # heads per core
DC = G * DK      # 256 per-core head dims
SCALE = 1.0 / np.sqrt(DK)  # 0.125
P = 128
NSC = 4          # 512-col s-chunks (query i-blocks and proj chunks)
SC = S // NSC    # 512
NJ = 16          # 128-row j chunks
ND = D // P      # 8 contraction chunks for projections

F32 = mybir.dt.float32
F32R = mybir.dt.float32r
BF16 = mybir.dt.bfloat16
I16 = mybir.dt.int16
FP16 = mybir.dt.float16
EXP = mybir.ActivationFunctionType.Exp
IDENT = mybir.ActivationFunctionType.Identity
COPY = mybir.ActivationFunctionType.Copy
ADD = mybir.AluOpType.add
MULT = mybir.AluOpType.mult

# Schraudolph exp -> fp16 bit pattern: u16 = round(s*SCALE*1024/ln2 + 15360 + c)
KMUL = float(SCALE * 1024.0 / np.log(2.0))
KADD = float(15360.0 - 45.0)

_NC_CACHE = None


def _build():
    nc = bacc.Bacc("TRN2", target_bir_lowering=False, debug=False)

    xq_t = nc.dram_tensor("xq_t", [D, S], BF16, kind="ExternalInput")
    xk_t = nc.dram_tensor("xk_t", [D, S], BF16, kind="ExternalInput")
    xv_t = nc.dram_tensor("xv_t", [D, S], BF16, kind="ExternalInput")
    wq_t = nc.dram_tensor("wq_t", [D, DC], BF16, kind="ExternalInput")
    wk_t = nc.dram_tensor("wk_t", [D, DC], BF16, kind="ExternalInput")
    wv_t = nc.dram_tensor("wv_t", [D, DC], BF16, kind="ExternalInput")
    wo_t = nc.dram_tensor("wo_t", [DC, D], BF16, kind="ExternalInput")
    bq_s = nc.dram_tensor("bq_s", [P, 2], F32, kind="ExternalInput")
    bk_s = nc.dram_tensor("bk_s", [P, 2], F32, kind="ExternalInput")
    y = nc.dram_tensor("y", [S, D], BF16, kind="ExternalOutput")

    # greedy two-engine load balancer for elementwise work
    ew = {"A": 0.0, "D": 0.0}

    def pick(cost_a, cost_d, force=None):
        if force is None:
            e = "A" if ew["A"] + cost_a <= ew["D"] + cost_d else "D"
        else:
            e = force
        ew[e] += cost_a if e == "A" else cost_d
        return e

    with tile.TileContext(nc) as tc:
        with (
            tc.tile_pool(name="persist", bufs=1) as persist,
            tc.tile_pool(name="xs", bufs=8) as xs,
            tc.tile_pool(name="stp", bufs=2, space="PSUM") as stp,
            tc.tile_pool(name="ub", bufs=2, space="PSUM") as ub,
            tc.tile_pool(name="scr", bufs=2, space="PSUM") as scr,
            tc.tile_pool(name="et", bufs=12) as etp,
            tc.tile_pool(name="rz", bufs=4) as rzp,
            tc.tile_pool(name="ysb", bufs=6) as ysb,
        ):
            # ---- persistent SBUF ----
            wq_sb = persist.tile([P, ND, DC], BF16)
            wk_sb = persist.tile([P, ND, DC], BF16)
            wv_sb = persist.tile([P, ND, DC], BF16)
            bq_sb = persist.tile([P, 2], F32)
            bk_sb = persist.tile([P, 2], F32)
            nc.sync.dma_start(
                out=wk_sb, in_=wk_t.ap().rearrange("(c p) m -> p c m", p=P)
            )
            nc.sync.dma_start(out=bk_sb, in_=bk_s.ap())
            nc.sync.dma_start(out=bq_sb, in_=bq_s.ap())
            woc_sb = persist.tile([P, 2, D], BF16)

            qt_sb = [persist.tile([P, S], F32R, name=f"qt{t}") for t in range(2)]
            kt_sb = [persist.tile([P, S], F32R, name=f"kt{t}") for t in range(2)]
            # vaug[p, jc, h, d]; d==64 is the ones column for Z
            vaug = persist.tile([P, NJ, G, 65], FP16)
            nc.vector.memset(vaug[:, :, :, 64:65], 1.0)
            o_sb = persist.tile([P, NSC * 4, DC], BF16)
            ot_sb = [persist.tile([P, S], BF16, name=f"ot{t}") for t in range(2)]
            ident_f = persist.tile([P, P], F32)
            make_identity(nc, ident_f)
            ident = persist.tile([P, P], BF16)
            nc.vector.tensor_copy(ident, ident_f)

            # ---- work items ----
            slabs = {}
            emitted = set()

            def dma_slab(key, xt, cb):
                # two s-halves per slab: consumers of the first half start
                # after half the transfer latency (subtile deps)
                t = xs.tile([P, ND, SC], BF16, tag="x", name="x")
                src_ap = xt.ap().rearrange("(c p) s -> p c s", p=P)
                for hf in range(2):
                    nc.sync.dma_start(
                        out=t[:, :, hf * HW_ : (hf + 1) * HW_],
                        in_=src_ap[
                            :, :, cb * SC + hf * HW_ : cb * SC + (hf + 1) * HW_
                        ],
                    )
                slabs[(key, cb)] = t
                emitted.add(("dma", key, cb))

            HW_ = SC // 2  # 256-col half for short PSUM holds

            def proj_qk(key, cb, mt, half, w_sb, b_sb, out_tiles):
                ps = scr.tile([P, HW_], F32, tag="scr", name="ps")
                slab = slabs[(key, cb)]
                for dc in range(ND):
                    nc.tensor.matmul(
                        ps,
                        w_sb[:, dc, mt * P : (mt + 1) * P],
                        slab[:, dc, half * HW_ : (half + 1) * HW_],
                        start=(dc == 0),
                        stop=(dc == ND - 1),
                    )
                dst = out_tiles[mt][:, cb * SC + half * HW_ : cb * SC + (half + 1) * HW_]
                e = pick(HW_ * 0.833 + 180, HW_ * 1.042 + 170)
                if e == "A":
                    nc.scalar.activation(dst, ps, IDENT, bias=b_sb[:, mt : mt + 1])
                else:
                    nc.vector.tensor_scalar(dst, ps, b_sb[:, mt : mt + 1], None, op0=ADD)
                emitted.add((key, cb, mt, half))

            def proj_v(jc):
                # j chunk jc (128 rows) -> vaug[:, jc, :, 0:64]
                cb, jq = divmod(jc, 4)
                ps = scr.tile([P, DC], F32, tag="scr", name="ps")
                slab = slabs[("v", cb)]
                for dc in range(ND):
                    nc.tensor.matmul(
                        ps,
                        slab[:, dc, jq * P : (jq + 1) * P],
                        wv_sb[:, dc, :],
                        start=(dc == 0),
                        stop=(dc == ND - 1),
                    )
                dst = vaug[:, jc, :, 0:64]
                src = ps.rearrange("p (h c) -> p h c", h=G)
                e = pick(DC * 0.833 + 180, DC * 1.042 + 170)
                if e == "A":
                    nc.scalar.activation(dst, src, COPY)
                else:
                    nc.vector.tensor_copy(dst, src)
                emitted.add(("v", jc))

            def transp2(ib, mt, pr):
                for it in range(ib * 4 + 2 * pr, ib * 4 + 2 * pr + 2):
                    tp = scr.tile([P, P], BF16, tag="scr", name="tp")
                    nc.tensor.transpose(tp, o_sb[:, it, mt * P : (mt + 1) * P], ident)
                    dst = ot_sb[mt][:, it * P : (it + 1) * P]
                    e = pick(P * 0.833 + 180, P * 0.521 + 170)
                    if e == "A":
                        nc.scalar.activation(dst, tp, COPY)
                    else:
                        nc.vector.tensor_copy(dst, tp)

            def oproj(it):
                # each half flies to HBM right after its stage copy, and the
                # two copies go to different engines: halves the tail chain
                ys = ysb.tile([P, D], BF16, tag="ysb", name="ysb")
                e0 = pick(SC * 0.833 + 180, SC * 1.042 + 170)
                for nh in range(2):
                    yp = scr.tile([P, SC], F32, tag="scr", name="yp")
                    for mt in range(2):
                        nc.tensor.matmul(
                            yp,
                            ot_sb[mt][:, it * P : (it + 1) * P],
                            woc_sb[:, mt, nh * SC : (nh + 1) * SC],
                            start=(mt == 0),
                            stop=(mt == 1),
                        )
                    dst = ys[:, nh * SC : (nh + 1) * SC]
                    e = e0 if nh == 0 else pick(
                        SC * 0.833 + 180, SC * 1.042 + 170,
                        force="D" if e0 == "A" else "A")
                    if e == "A":
                        nc.scalar.activation(dst, yp, COPY)
                    else:
                        nc.vector.tensor_copy(dst, yp)
                    nc.sync.dma_start(
                        out=y.ap()[it * P : (it + 1) * P, nh * SC : (nh + 1) * SC],
                        in_=dst,
                    )

            def phase_c(ib):
                items = [
                    lambda mt=mt, pr=pr: transp2(ib, mt, pr)
                    for mt in range(2)
                    for pr in range(2)
                ]
                items += [lambda it=it: oproj(it) for it in range(ib * 4, ib * 4 + 4)]
                return items

            def phase_c_late(ib):
                items = []
                for pr in range(2):
                    items.append(lambda pr=pr: transp2(ib, 1, pr))
                    for it in range(ib * 4 + 2 * pr, ib * 4 + 2 * pr + 2):
                        items.append(lambda it=it: oproj(it))
                return items

            work = deque()
            light = deque()

            def drain(tick):
                if light:
                    light.popleft()()
                if work:
                    work.popleft()()

            def drain_until(key):
                while key not in emitted:
                    assert work or light, f"work exhausted before {key}"
                    if light:
                        light.popleft()()
                    elif work:
                        work.popleft()()

            # ---- pipeline fill: K/Q/V chunk 0 ----
            dma_slab("k", xk_t, 0)
            proj_qk("k", 0, 0, 0, wk_sb, bk_sb, kt_sb)
            proj_qk("k", 0, 0, 1, wk_sb, bk_sb, kt_sb)
            nc.sync.dma_start(
                out=wq_sb, in_=wq_t.ap().rearrange("(c p) m -> p c m", p=P)
            )
            dma_slab("q", xq_t, 0)
            proj_qk("q", 0, 0, 0, wq_sb, bq_sb, qt_sb)
            proj_qk("q", 0, 0, 1, wq_sb, bq_sb, qt_sb)
            nc.sync.dma_start(out=wv_sb, in_=wv_t.ap().rearrange("(c p) m -> p c m", p=P))
            dma_slab("v", xv_t, 0)

            def load_woc():
                nc.sync.dma_start(
                    out=woc_sb, in_=wo_t.ap().rearrange("(t p) n -> p t n", p=P)
                )

            work.append(lambda: proj_qk("k", 0, 1, 0, wk_sb, bk_sb, kt_sb))
            work.append(lambda: proj_qk("k", 0, 1, 1, wk_sb, bk_sb, kt_sb))
            work.append(lambda: proj_qk("q", 0, 1, 0, wq_sb, bq_sb, qt_sb))
            work.append(lambda: proj_qk("q", 0, 1, 1, wq_sb, bq_sb, qt_sb))
            for jc in range(4):
                work.append(lambda jc=jc: proj_v(jc))
            # K slabs lead V slabs in the DMA queue: ib0's score stream
            # consumes K chunks at nearly full DMA bandwidth
            light.append(lambda: dma_slab("k", xk_t, 1))
            light.append(lambda: dma_slab("v", xv_t, 1))
            light.append(lambda: dma_slab("k", xk_t, 2))
            light.append(lambda: dma_slab("k", xk_t, 3))
            light.append(lambda: dma_slab("v", xv_t, 2))
            light.append(lambda: dma_slab("v", xv_t, 3))
            for cb in range(1, NSC):
                for half in range(2):
                    work.append(
                        lambda cb=cb, half=half: proj_qk("k", cb, 0, half, wk_sb, bk_sb, kt_sb)
                    )
                    work.append(
                        lambda cb=cb, half=half: proj_qk("k", cb, 1, half, wk_sb, bk_sb, kt_sb)
                    )
                for jc in range(cb * 4, cb * 4 + 4):
                    work.append(lambda jc=jc: proj_v(jc))
            light.append(load_woc)
            for cb in range(1, NSC):
                light.append(lambda cb=cb: dma_slab("q", xq_t, cb))
                for half in range(2):
                    work.append(
                        lambda cb=cb, half=half: proj_qk("q", cb, 0, half, wq_sb, bq_sb, qt_sb)
                    )
                    work.append(
                        lambda cb=cb, half=half: proj_qk("q", cb, 1, half, wq_sb, bq_sb, qt_sb)
                    )

            # ---- attention ticks: (ib, hp, J); head pair hp, 128-row j chunk J
            seq = [(ib, hp, J) for ib in range(NSC) for hp in range(2) for J in range(NJ)]
            u_tiles = {}
            et_tiles = {}

            def emit_st_exp(idx):
                ib, hp, J = seq[idx]
                if J == 0:
                    drain_until(("q", ib, hp, 0))
                    drain_until(("q", ib, hp, 1))
                if ib == 0:
                    drain_until(("k", J // 4, hp, 0))
                    drain_until(("k", J // 4, hp, 1))
                st = stp.tile([P, 2 * SC], F32, tag="st", name="st")
                for hx in range(2):
                    nc.tensor.matmul(
                        st[:, hx * SC : (hx + 1) * SC],
                        kt_sb[hp][hx * DK : (hx + 1) * DK, J * P : (J + 1) * P],
                        qt_sb[hp][hx * DK : (hx + 1) * DK, ib * SC : (ib + 1) * SC],
                        start=True,
                        stop=True,
                        tile_position=(hx * DK, 0),
                    )
                et = etp.tile([P, 2, SC], I16, tag="et", name="et")
                et_flat = et.rearrange("p a b -> p (a b)")
                # strict A/D alternation keeps the two st pipelines decoupled
                e = pick(2 * SC * 0.833 + 210, 2 * SC * 1.042 + 170,
                         force="A" if idx % 2 == 0 else "D")
                if e == "A":
                    nc.scalar.activation(et_flat.bitcast(FP16), st, EXP, scale=float(SCALE))
                else:
                    nc.vector.tensor_scalar(et_flat, st, KMUL, KADD, op0=MULT, op1=ADD)
                et_tiles[idx] = et

            def emit_pv(idx):
                ib, hp, J = seq[idx]
                if J == 0:
                    for hx in range(2):
                        u_tiles[(hp, hx)] = ub.tile([P, 4, 65], F32, tag="u", name="u")
                if ib == 0 and hp == 0:
                    drain_until(("v", J))
                et = et_tiles.pop(idx).bitcast(FP16)
                for hx in range(2):
                    u = u_tiles[(hp, hx)]
                    for it in range(4):
                        nc.tensor.matmul(
                            u[:, it, :],
                            et[:, hx, it * P : (it + 1) * P],
                            vaug[:, J, 2 * hp + hx, :],
                            start=(J == 0 and it == 0),
                            stop=(J == NJ - 1 and it == 3),
                            skip_group_check=True,
                            tile_position=(0, 0),
                        )
                if J == NJ - 1:
                    finish_pair(ib, hp)

            def finish_pair(ib, hp):
                for hx in range(2):
                    u = u_tiles.pop((hp, hx))
                    h = 2 * hp + hx
                    rz = rzp.tile([P, 4, 1], F32, tag="rz", name="rz")
                    nc.vector.reciprocal(rz, u[:, :, 64:65])
                    ew["D"] += 4 * 1.042 + 170
                    for it in range(4):
                        dst = o_sb[:, ib * 4 + it, h * DK : (h + 1) * DK]
                        e = pick(DK * 0.833 + 180, DK * 1.042 + 170,
                                 force="A" if (it + hx) % 2 == 0 else "D")
                        if e == "A":
                            nc.scalar.activation(dst, u[:, it, 0:DK], COPY, scale=rz[:, it])
                        else:
                            nc.vector.tensor_scalar(dst, u[:, it, 0:DK], rz[:, it], None, op0=MULT)
                if hp == 0 and ib == NSC - 1:
                    work.extend(
                        [lambda pr=pr: transp2(NSC - 1, 0, pr) for pr in range(2)]
                    )
                elif hp == 1:
                    items = phase_c(ib) if ib < NSC - 1 else phase_c_late(ib)
                    work.extend(items)

            SKEW = 4  # PV lags scores/exp so its matmuls never park in the
            # PE wait queue (depth 4) blocking later scores
            for idx in range(len(seq) + SKEW):
                if idx < len(seq):
                    emit_st_exp(idx)
                if idx >= SKEW:
                    emit_pv(idx - SKEW)
                drain(idx)

            while work or light:
                (light or work).popleft()()

    nc.compile()
    return nc


def _get_nc():
    global _NC_CACHE
    if _NC_CACHE is None:
        _NC_CACHE = _build()
    return _NC_CACHE


def _in_maps(query, key, value, wq, wk, wv, wo, bq, bk):
    bf = ml_dtypes.bfloat16
    maps = []
    for c in range(8):
        b, g = divmod(c, 4)
        sl = slice(g * DC, (g + 1) * DC)
        maps.append(
            {
                "xq_t": np.ascontiguousarray(query[:, b, :].T).astype(bf),
                "xk_t": np.ascontiguousarray(key[:, b, :].T).astype(bf),
                "xv_t": np.ascontiguousarray(value[:, b, :].T).astype(bf),
                "wq_t": np.ascontiguousarray(wq[sl, :].T).astype(bf),
                "wk_t": np.ascontiguousarray(wk[sl, :].T).astype(bf),
                "wv_t": np.ascontiguousarray(wv[sl, :].T).astype(bf),
                "wo_t": np.ascontiguousarray(wo[:, sl].T).astype(bf),
                "bq_s": np.ascontiguousarray(bq[sl].reshape(2, P).T),
                "bk_s": np.ascontiguousarray(bk[sl].reshape(2, P).T),
            }
        )
    return maps


def kernel(
    query, key, value, wq, bq, wk, bk, wv, bv, wo, bo, **_kw
) -> np.ndarray:
    query = np.asarray(query, np.float32)
    key = np.asarray(key, np.float32)
    value = np.asarray(value, np.float32)
    wq = np.asarray(wq, np.float32)
    wk = np.asarray(wk, np.float32)
    wv = np.asarray(wv, np.float32)
    wo = np.asarray(wo, np.float32)
    bq = np.asarray(bq, np.float32)
    bk = np.asarray(bk, np.float32)
    bv = np.asarray(bv, np.float32)
    bo = np.asarray(bo, np.float32)

    nc = _get_nc()
    res = run_bass_kernel_spmd(
        nc, _in_maps(query, key, value, wq, wk, wv, wo, bq, bk),
        core_ids=list(range(8)),
    )

    out = np.zeros((S, B, D), np.float32)
    for c in range(8):
        out[:, c // 4, :] += res.results[c]["y"].astype(np.float32)
    out += bo + wo @ bv
    return out
